# revision 2
# baseline (speedup 1.0000x reference)
"""Self-contained Trainium2 kernel for nn_Linear_14293651161742.

Computes y = act_dequant(act_quant(x)) @ (weight * expand(w_scale))^T which is
mathematically y[m,o] = sum_k x[m,k] * weight[o,k] * w_scale[o//128, k//128]
(the act_quant divide/multiply round-trip is an exact no-op up to fp32
rounding, far below the bf16 matmul noise floor).

Strategy: shard M across the 8 cores (column of the hint is worse: it
replicates the 128 MiB x per core; M-sharding needs only 96 MiB/core of HBM
traffic, leaving the kernel compute-bound at the bf16 PE roofline).

Host does layout prep only (transposes / scale replication); all arithmetic
(dequant, casts, GEMM) runs on device.
"""

import sys

if "/opt/trn_rl_repo" not in sys.path:
    sys.path.insert(0, "/opt/trn_rl_repo")

import numpy as np

import concourse.bacc as bacc
import concourse.mybir as mybir
import concourse.tile as tile
from concourse import bass_utils

P = 128
N_CORES = 8

F32 = mybir.dt.float32
BF16 = mybir.dt.bfloat16


def build_gemm_nc(M_loc: int, K: int, O: int):
    """Per-core program: yt[O, M_loc] = (wt * scale)^T-contracted with xt.

    Inputs (per core):
      xt  [K, M_loc] f32 : x slice, K-major (pre-transposed on host)
      wt  [K, O]     f32 : full weight, K-major (pre-transposed on host)
      ws  [P, K//P, O//P] f32 : w_scale[ob, kb] replicated across partitions,
                                indexed [p, kb, ob]
    Output:
      yt  [O, M_loc] f32 : y^T slice (host transposes back)
    """
    KT = K // P            # k tiles
    OB = O // P            # 128-wide o tiles
    OCW = 512              # o-chunk width (psum partition groups per chunk: OCW/P)
    OC = O // OCW          # o chunks
    JT = OCW // P          # o tiles per chunk
    MCW = min(512, M_loc)  # matmul moving free dim
    MC = M_loc // MCW      # m chunks

    nc = bacc.Bacc("TRN2", target_bir_lowering=False, debug=False)
    xt = nc.dram_tensor("xt", [K, M_loc], F32, kind="ExternalInput")
    wt = nc.dram_tensor("wt", [K, O], F32, kind="ExternalInput")
    ws = nc.dram_tensor("ws", [P, KT, OB], F32, kind="ExternalInput")
    yt = nc.dram_tensor("yt", [O, M_loc], F32, kind="ExternalOutput")

    xt_r = xt.ap().rearrange("(kt p) m -> p kt m", p=P)    # [P, KT, M_loc]
    wt_r = wt.ap().rearrange("(kt p) o -> p kt o", p=P)    # [P, KT, O]
    yt_r = yt.ap().rearrange("(ot p) m -> p ot m", p=P)    # [P, OB, M_loc]

    with tile.TileContext(nc) as tc:
        with (
            tc.tile_pool(name="const", bufs=1) as const_pool,
            tc.tile_pool(name="xstage", bufs=3) as xstage_pool,
            tc.tile_pool(name="xbf", bufs=1) as xbf_pool,
            tc.tile_pool(name="wstage", bufs=3) as wstage_pool,
            tc.tile_pool(name="wbf", bufs=2) as wbf_pool,
            tc.tile_pool(name="yout", bufs=4) as y_pool,
            tc.tile_pool(name="psum", bufs=1, space="PSUM") as psum_pool,
        ):
            ws_sb = const_pool.tile([P, KT, OB], F32)
            nc.sync.dma_start(ws_sb[:], ws.ap())

            # Prologue: stream x in, cast to resident bf16. Interleave the
            # first w chunk's load+dequant per k-tile so the PE can start
            # almost immediately (fine-grained deps via per-kt tiles).
            x_bf = []
            for kt in range(KT):
                xs = xstage_pool.tile([P, M_loc], F32, tag="xs")
                nc.sync.dma_start(xs[:], xt_r[:, kt])
                xb = xbf_pool.tile([P, M_loc], BF16, tag=f"xb{kt}")
                nc.vector.tensor_copy(xb[:], xs[:])
                x_bf.append(xb)

            def load_dequant_chunk(oc):
                """Load + dequant w chunk oc -> list of KT bf16 [P, OCW] tiles."""
                w_bf = []
                for kt in range(KT):
                    wst = wstage_pool.tile([P, OCW], F32, tag="wst")
                    nc.sync.dma_start(wst[:], wt_r[:, kt, oc * OCW:(oc + 1) * OCW])
                    wb = wbf_pool.tile([P, OCW], BF16, tag=f"wb{kt}")
                    nc.vector.tensor_tensor(
                        wb.rearrange("p (g j) -> p g j", j=P),
                        wst.rearrange("p (g j) -> p g j", j=P),
                        ws_sb[:, kt, oc * JT:(oc + 1) * JT, None].to_broadcast(
                            [P, JT, P]
                        ),
                        mybir.AluOpType.mult,
                    )
                    w_bf.append(wb)
                return w_bf

            for oc in range(OC):
                w_bf = load_dequant_chunk(oc)
                # one psum bank per (j, mc); k-outer keeps the PE dense
                psums = {}
                for j in range(JT):
                    for mc in range(MC):
                        psums[(j, mc)] = psum_pool.tile(
                            [P, MCW], F32, tag=f"ps{j}_{mc}", name=f"ps{j}_{mc}"
                        )
                for kt in range(KT):
                    for j in range(JT):
                        lhsT = w_bf[kt][:, j * P:(j + 1) * P]
                        for mc in range(MC):
                            nc.tensor.matmul(
                                psums[(j, mc)][:],
                                lhsT,
                                x_bf[kt][:, mc * MCW:(mc + 1) * MCW],
                                start=(kt == 0),
                                stop=(kt == KT - 1),
                            )
                for j in range(JT):
                    for mc in range(MC):
                        yo = y_pool.tile([P, MCW], F32, tag="yo")
                        nc.any.tensor_copy(yo[:], psums[(j, mc)][:])
                        nc.sync.dma_start(
                            yt_r[:, oc * JT + j, mc * MCW:(mc + 1) * MCW],
                            yo[:],
                        )
    nc.compile()
    return nc


_CACHED = {}


def _get_nc(M_loc, K, O):
    key = (M_loc, K, O)
    if key not in _CACHED:
        _CACHED[key] = build_gemm_nc(M_loc, K, O)
    return _CACHED[key]


def kernel(x: np.ndarray, weight: np.ndarray, w_scale: np.ndarray) -> np.ndarray:
    M, K = x.shape
    O = weight.shape[0]
    assert M % N_CORES == 0
    M_loc = M // N_CORES
    KT, OB = K // P, O // P

    nc = _get_nc(M_loc, K, O)

    wt = np.ascontiguousarray(weight.T)                       # [K, O]
    ws_rep = np.ascontiguousarray(
        np.broadcast_to(w_scale.T[None], (P, KT, OB))
    ).astype(np.float32)

    in_maps = []
    for c in range(N_CORES):
        xt_c = np.ascontiguousarray(x[c * M_loc:(c + 1) * M_loc, :].T)  # [K, M_loc]
        in_maps.append({"xt": xt_c, "wt": wt, "ws": ws_rep})

    res = bass_utils.run_bass_kernel_spmd(
        nc, in_maps, core_ids=list(range(N_CORES))
    )
    return np.concatenate(
        [np.ascontiguousarray(res.results[c]["yt"].T) for c in range(N_CORES)],
        axis=0,
    )


# revision 3
# speedup vs baseline: 1.0987x; 1.0987x over previous
"""Self-contained Trainium2 kernel for nn_Linear_14293651161742.

Computes y = act_dequant(act_quant(x)) @ (weight * expand(w_scale))^T which is
mathematically y[m,o] = sum_k x[m,k] * weight[o,k] * w_scale[o//128, k//128]
(the act_quant divide/multiply round-trip is an exact no-op up to fp32
rounding, far below the bf16 matmul noise floor).

Strategy: shard M across the 8 cores (column of the hint is worse: it
replicates the 128 MiB x per core; M-sharding needs only 96 MiB/core of HBM
traffic, leaving the kernel compute-bound at the bf16 PE roofline).

Host does layout prep only (transposes / scale replication); all arithmetic
(dequant, casts, GEMM) runs on device.
"""

import sys

if "/opt/trn_rl_repo" not in sys.path:
    sys.path.insert(0, "/opt/trn_rl_repo")

import numpy as np

import concourse.bacc as bacc
import concourse.mybir as mybir
import concourse.tile as tile
from concourse import bass_utils

P = 128
N_CORES = 8

F32 = mybir.dt.float32
BF16 = mybir.dt.bfloat16


def build_gemm_nc(M_loc: int, K: int, O: int):
    """Per-core program: yt[O, M_loc] = (wt * scale)^T-contracted with xt.

    Inputs (per core):
      xt  [K, M_loc] f32 : x slice, K-major (pre-transposed on host)
      wt  [K, O]     f32 : full weight, K-major (pre-transposed on host)
      ws  [P, K//P, O//P] f32 : w_scale[ob, kb] replicated across partitions,
                                indexed [p, kb, ob]
    Output:
      yt  [O, M_loc] f32 : y^T slice (host transposes back)
    """
    KT = K // P            # k tiles
    OB = O // P            # 128-wide o tiles
    OCW = 512              # o-chunk width (psum partition groups per chunk: OCW/P)
    OC = O // OCW          # o chunks
    JT = OCW // P          # o tiles per chunk
    MCW = min(512, M_loc)  # matmul moving free dim
    MC = M_loc // MCW      # m chunks

    nc = bacc.Bacc("TRN2", target_bir_lowering=False, debug=False)
    xt = nc.dram_tensor("xt", [K, M_loc], F32, kind="ExternalInput")
    wt = nc.dram_tensor("wt", [K, O], F32, kind="ExternalInput")
    ws = nc.dram_tensor("ws", [P, KT, OB], F32, kind="ExternalInput")
    yt = nc.dram_tensor("yt", [O, M_loc], F32, kind="ExternalOutput")

    xt_r = xt.ap().rearrange("(kt p) m -> p kt m", p=P)    # [P, KT, M_loc]
    wt_r = wt.ap().rearrange("(kt p) o -> p kt o", p=P)    # [P, KT, O]
    yt_r = yt.ap().rearrange("(ot p) m -> p ot m", p=P)    # [P, OB, M_loc]

    XG = 2 if KT % 2 == 0 else 1     # k-tiles per x-load DMA (~1 MiB)
    WG = 4 if KT % 4 == 0 else 1     # k-tiles per w-load DMA (~1 MiB)

    with tile.TileContext(nc) as tc:
        with (
            tc.tile_pool(name="const", bufs=1) as const_pool,
            tc.tile_pool(name="xstage", bufs=2) as xstage_pool,
            tc.tile_pool(name="xbf", bufs=1) as xbf_pool,
            tc.tile_pool(name="wstage", bufs=2) as wstage_pool,
            tc.tile_pool(name="wbf", bufs=2) as wbf_pool,
            tc.tile_pool(name="yout", bufs=1) as y_pool,
            tc.tile_pool(name="psum", bufs=1, space="PSUM") as psum_pool,
        ):
            ws_sb = const_pool.tile([P, KT, OB], F32)
            nc.sync.dma_start(ws_sb[:], ws.ap())

            x_bf = [None] * KT
            w_chunks = {}  # oc -> list of KT bf16 [P, OCW] tiles

            def emit_x_group(g):
                xs = xstage_pool.tile([P, XG, M_loc], F32, tag="xs", name="xs")
                nc.sync.dma_start(xs[:], xt_r[:, g * XG:(g + 1) * XG, :])
                for i in range(XG):
                    kt = g * XG + i
                    xb = xbf_pool.tile([P, M_loc], BF16, tag=f"xb{kt}",
                                       name=f"xb{kt}")
                    nc.vector.tensor_copy(xb[:], xs[:, i])
                    x_bf[kt] = xb

            def emit_w_group(oc, g):
                wst = wstage_pool.tile([P, WG, OCW], F32, tag="wst", name="wst")
                nc.sync.dma_start(
                    wst[:], wt_r[:, g * WG:(g + 1) * WG, oc * OCW:(oc + 1) * OCW]
                )
                for i in range(WG):
                    kt = g * WG + i
                    wb = wbf_pool.tile([P, OCW], BF16, tag=f"wb{kt}",
                                       name=f"wb{kt}")
                    nc.vector.tensor_tensor(
                        wb.rearrange("p (g j) -> p g j", j=P),
                        wst[:, i].rearrange("p (g j) -> p g j", j=P),
                        ws_sb[:, kt, oc * JT:(oc + 1) * JT, None].to_broadcast(
                            [P, JT, P]
                        ),
                        mybir.AluOpType.mult,
                    )
                    w_chunks[oc][kt] = wb

            # Prologue: interleave w-chunk-0 and x loads (SP FIFO order decides
            # arrival); first matmul only needs x[0] + w0[0] -> starts ~6us in.
            w_chunks[0] = [None] * KT
            for g in range(KT // XG):
                if g * XG % WG == 0:
                    emit_w_group(0, g * XG // WG)
                emit_x_group(g)

            for oc in range(OC):
                # prefetch next w chunk ahead of this chunk's matmuls in
                # program order (SP queue: never behind compute-gated work)
                if oc + 1 < OC:
                    w_chunks[oc + 1] = [None] * KT
                    for g in range(KT // WG):
                        emit_w_group(oc + 1, g)
                w_bf = w_chunks[oc]
                psums = {}
                for j in range(JT):
                    for mc in range(MC):
                        psums[(j, mc)] = psum_pool.tile(
                            [P, MCW], F32, tag=f"ps{j}_{mc}", name=f"ps{j}_{mc}"
                        )
                for kt in range(KT):
                    for j in range(JT):
                        lhsT = w_bf[kt][:, j * P:(j + 1) * P]
                        for mc in range(MC):
                            nc.tensor.matmul(
                                psums[(j, mc)][:],
                                lhsT,
                                x_bf[kt][:, mc * MCW:(mc + 1) * MCW],
                                start=(kt == 0),
                                stop=(kt == KT - 1),
                            )
                # evict on DVE (fast); gather per mc, store 1 MiB on ACT ring
                for mc in range(MC):
                    ysb = y_pool.tile([P, JT, MCW], F32, tag=f"ysb{mc}",
                                      name=f"ysb{mc}")
                    for j in range(JT):
                        nc.vector.tensor_copy(ysb[:, j], psums[(j, mc)][:])
                    nc.scalar.dma_start(
                        yt_r[:, oc * JT:(oc + 1) * JT,
                             mc * MCW:(mc + 1) * MCW],
                        ysb[:],
                    )
                del w_chunks[oc]
    nc.compile()
    return nc


_CACHED = {}


def _get_nc(M_loc, K, O):
    key = (M_loc, K, O)
    if key not in _CACHED:
        _CACHED[key] = build_gemm_nc(M_loc, K, O)
    return _CACHED[key]


def kernel(x: np.ndarray, weight: np.ndarray, w_scale: np.ndarray) -> np.ndarray:
    M, K = x.shape
    O = weight.shape[0]
    assert M % N_CORES == 0
    M_loc = M // N_CORES
    KT, OB = K // P, O // P

    nc = _get_nc(M_loc, K, O)

    wt = np.ascontiguousarray(weight.T)                       # [K, O]
    ws_rep = np.ascontiguousarray(
        np.broadcast_to(w_scale.T[None], (P, KT, OB))
    ).astype(np.float32)

    in_maps = []
    for c in range(N_CORES):
        xt_c = np.ascontiguousarray(x[c * M_loc:(c + 1) * M_loc, :].T)  # [K, M_loc]
        in_maps.append({"xt": xt_c, "wt": wt, "ws": ws_rep})

    res = bass_utils.run_bass_kernel_spmd(
        nc, in_maps, core_ids=list(range(N_CORES))
    )
    return np.concatenate(
        [np.ascontiguousarray(res.results[c]["yt"].T) for c in range(N_CORES)],
        axis=0,
    )


# revision 4
# speedup vs baseline: 1.1481x; 1.0450x over previous
"""Self-contained Trainium2 kernel for nn_Linear_14293651161742.

Computes y = act_dequant(act_quant(x)) @ (weight * expand(w_scale))^T which is
mathematically y[m,o] = sum_k x[m,k] * weight[o,k] * w_scale[o//128, k//128]
(the act_quant divide/multiply round-trip is an exact no-op up to fp32
rounding, far below the bf16 matmul noise floor).

Strategy: shard M across the 8 cores (column of the hint is worse: it
replicates the 128 MiB x per core; M-sharding needs only 96 MiB/core of HBM
traffic, leaving the kernel compute-bound at the bf16 PE roofline).

Host does layout prep only (transposes / scale replication); all arithmetic
(dequant, casts, GEMM) runs on device.
"""

import sys

if "/opt/trn_rl_repo" not in sys.path:
    sys.path.insert(0, "/opt/trn_rl_repo")

import numpy as np

import concourse.bacc as bacc
import concourse.mybir as mybir
import concourse.tile as tile
from concourse import bass_utils

P = 128
N_CORES = 8

F32 = mybir.dt.float32
BF16 = mybir.dt.bfloat16


def build_gemm_nc(M_loc: int, K: int, O: int):
    """Per-core program: yt[O, M_loc] = (wt * scale)^T-contracted with xt.

    Inputs (per core):
      xt  [K, M_loc] f32 : x slice, K-major (pre-transposed on host)
      wt  [K, O]     f32 : full weight, K-major (pre-transposed on host)
      ws  [P, K//P, O//P] f32 : w_scale[ob, kb] replicated across partitions,
                                indexed [p, kb, ob]
    Output:
      yt  [O, M_loc] f32 : y^T slice (host transposes back)
    """
    KT = K // P            # k tiles
    OB = O // P            # 128-wide o tiles
    OCW = 512              # o-chunk width (psum partition groups per chunk: OCW/P)
    OC = O // OCW          # o chunks
    JT = OCW // P          # o tiles per chunk
    MCW = min(512, M_loc)  # matmul moving free dim
    MC = M_loc // MCW      # m chunks

    nc = bacc.Bacc("TRN2", target_bir_lowering=False, debug=False)
    xt = nc.dram_tensor("xt", [K, M_loc], F32, kind="ExternalInput")
    wt = nc.dram_tensor("wt", [K, O], F32, kind="ExternalInput")
    ws = nc.dram_tensor("ws", [P, KT, OB], F32, kind="ExternalInput")
    yt = nc.dram_tensor("yt", [O, M_loc], F32, kind="ExternalOutput")

    xt_r = xt.ap().rearrange("(kt p) m -> p kt m", p=P)    # [P, KT, M_loc]
    wt_r = wt.ap().rearrange("(kt p) o -> p kt o", p=P)    # [P, KT, O]
    yt_r = yt.ap().rearrange("(ot p) m -> p ot m", p=P)    # [P, OB, M_loc]

    with tile.TileContext(nc) as tc:
        with (
            tc.tile_pool(name="const", bufs=1) as const_pool,
            tc.tile_pool(name="xstage", bufs=6) as xstage_pool,
            tc.tile_pool(name="xbf", bufs=1) as xbf_pool,
            tc.tile_pool(name="wstage", bufs=8) as wstage_pool,
            tc.tile_pool(name="wbf", bufs=2) as wbf_pool,
            tc.tile_pool(name="yout", bufs=1) as y_pool,
            tc.tile_pool(name="psum", bufs=1, space="PSUM") as psum_pool,
        ):
            ws_sb = const_pool.tile([P, KT, OB], F32)
            nc.sync.dma_start(ws_sb[:], ws.ap())

            x_bf = [None] * KT
            w_chunks = {}  # oc -> list of KT bf16 [P, OCW] tiles

            # Per-kt staging with many bufs: the DMA for slot reuse only
            # waits on a cast/dequant several tiles back, keeping many DMAs
            # in flight (shallow bufs serialize DMA behind in-order DVE).
            def emit_x_load(kt):
                xs = xstage_pool.tile([P, M_loc], F32, tag="xs", name="xs")
                nc.sync.dma_start(xs[:], xt_r[:, kt, :])
                xb = xbf_pool.tile([P, M_loc], BF16, tag=f"xb{kt}",
                                   name=f"xb{kt}")
                nc.vector.tensor_copy(xb[:], xs[:])
                x_bf[kt] = xb

            def emit_w_load(oc, kt):
                wst = wstage_pool.tile([P, OCW], F32, tag="wst", name="wst")
                nc.sync.dma_start(wst[:], wt_r[:, kt, oc * OCW:(oc + 1) * OCW])
                wb = wbf_pool.tile([P, OCW], BF16, tag=f"wb{kt}",
                                   name=f"wb{kt}")
                nc.vector.tensor_tensor(
                    wb.rearrange("p (g j) -> p g j", j=P),
                    wst.rearrange("p (g j) -> p g j", j=P),
                    ws_sb[:, kt, oc * JT:(oc + 1) * JT, None].to_broadcast(
                        [P, JT, P]
                    ),
                    mybir.AluOpType.mult,
                )
                w_chunks[oc][kt] = wb

            # Prologue: interleave w-chunk-0 and x loads (SP FIFO order decides
            # arrival); first matmul only needs x[0] + w0[0] -> starts early.
            w_chunks[0] = [None] * KT
            for kt in range(KT):
                emit_w_load(0, kt)
                emit_x_load(kt)

            for oc in range(OC):
                # prefetch next w chunk ahead of this chunk's matmuls in
                # program order (SP queue: never behind compute-gated work)
                if oc + 1 < OC:
                    w_chunks[oc + 1] = [None] * KT
                    for kt in range(KT):
                        emit_w_load(oc + 1, kt)
                w_bf = w_chunks[oc]
                psums = {}
                for j in range(JT):
                    for mc in range(MC):
                        psums[(j, mc)] = psum_pool.tile(
                            [P, MCW], F32, tag=f"ps{j}_{mc}", name=f"ps{j}_{mc}"
                        )
                for kt in range(KT):
                    for j in range(JT):
                        lhsT = w_bf[kt][:, j * P:(j + 1) * P]
                        for mc in range(MC):
                            nc.tensor.matmul(
                                psums[(j, mc)][:],
                                lhsT,
                                x_bf[kt][:, mc * MCW:(mc + 1) * MCW],
                                start=(kt == 0),
                                stop=(kt == KT - 1),
                            )
                # evict on DVE (fast); gather per mc, store 1 MiB on ACT ring
                for mc in range(MC):
                    ysb = y_pool.tile([P, JT, MCW], F32, tag=f"ysb{mc}",
                                      name=f"ysb{mc}")
                    for j in range(JT):
                        nc.vector.tensor_copy(ysb[:, j], psums[(j, mc)][:])
                    nc.scalar.dma_start(
                        yt_r[:, oc * JT:(oc + 1) * JT,
                             mc * MCW:(mc + 1) * MCW],
                        ysb[:],
                    )
                del w_chunks[oc]
    nc.compile()
    return nc


_CACHED = {}


def _get_nc(M_loc, K, O):
    key = (M_loc, K, O)
    if key not in _CACHED:
        _CACHED[key] = build_gemm_nc(M_loc, K, O)
    return _CACHED[key]


def kernel(x: np.ndarray, weight: np.ndarray, w_scale: np.ndarray) -> np.ndarray:
    M, K = x.shape
    O = weight.shape[0]
    assert M % N_CORES == 0
    M_loc = M // N_CORES
    KT, OB = K // P, O // P

    nc = _get_nc(M_loc, K, O)

    wt = np.ascontiguousarray(weight.T)                       # [K, O]
    ws_rep = np.ascontiguousarray(
        np.broadcast_to(w_scale.T[None], (P, KT, OB))
    ).astype(np.float32)

    in_maps = []
    for c in range(N_CORES):
        xt_c = np.ascontiguousarray(x[c * M_loc:(c + 1) * M_loc, :].T)  # [K, M_loc]
        in_maps.append({"xt": xt_c, "wt": wt, "ws": ws_rep})

    res = bass_utils.run_bass_kernel_spmd(
        nc, in_maps, core_ids=list(range(N_CORES))
    )
    return np.concatenate(
        [np.ascontiguousarray(res.results[c]["yt"].T) for c in range(N_CORES)],
        axis=0,
    )
